# revision 95
# baseline (speedup 1.0000x reference)
"""Multi-head self-attention with RoPE on 8 Trainium2 NeuronCores.

Sharding: data-parallel over batch (2) x tensor-parallel over heads
(16 heads -> 4 groups of 4). Core c handles batch c//4, head group c%4.
Each core computes a partial output projection (d_in-sharded wo); the
4 partials per batch are summed on the host (the unshard step).

Optimized around the PE p-state ramp: the PE runs at 2.4 GHz only after
~3us of continuous busy, so the schedule keeps an uninterrupted PE
instruction stream:
  - Projections (fp32r) and attention (bf16) use SEPARATE psum rings so
    projection matmuls for chunk b+1 are always ready as filler while
    the attention exp pipeline (ACT engine) catches up.
  - Scores for a head pair land in one 2-bank psum tile [128, 2, 512],
    so each k-tile needs a single merged Exp activation.
  - Attention tensors (Q/K post-rope, V, exp scores, normalized A, wo)
    are bf16: <256-col band matmuls run at 1 cyc/row, DVE ops get 2x/4x
    modes, LDWEIGHTS halves.
  - Emission order is software-pipelined: B(j) attention, then A(j+1)
    projections, then C(j) output projection; the tile scheduler fills
    PE stalls with whatever is ready.
Per-core math identical to the baseline: scores computed transposed
S[k, q] = K' Q'^T; V carries a ones column so the attention-out matmul
accumulates the softmax denominator for free; no max-subtraction.
"""

import sys

for _p in ("/opt/trn_rl_repo", "/opt/pypackages"):
    if _p not in sys.path:
        sys.path.append(_p)

import numpy as np
import ml_dtypes

import concourse.bass as bass
from concourse.bass import _add_dep_helper
import concourse.mybir as mybir
import concourse.tile as tile
from concourse import bacc
from concourse.bass_utils import run_bass_kernel_spmd

# Problem constants (hardcoded per contract)
B = 2
S = 2048
DM = 1024
NH = 16
DK = 64
THETA = 10000.0
N_CORES = 8
HG = 4            # head groups (tensor-parallel)
HL = NH // HG     # heads per core = 4
DG = HL * DK      # group out dim = 256

P = 128
KO = DM // P      # 8 contraction subtiles for projections
MT = 2            # 128-row tiles of the 256-wide Q/K head-group dim
QB = 512          # q block width
NQB = S // QB     # 4
NKT = S // P      # 16 k tiles
F32 = mybir.dt.float32
F32R = mybir.dt.float32r
BF16 = mybir.dt.bfloat16
EXP = mybir.ActivationFunctionType.Exp


def _emit(ctx, tc, d):
    nc = tc.nc
    # PSUM: spp (scores, [128,2,512] = 2 banks) x2 = 4 banks;
    # ops0/ops1 (attention accumulators + oproj outputs) = 2 banks;
    # pp (projection transients) x2 = 2 banks. Total 8.
    const = ctx.enter_context(tc.tile_pool(name="const", bufs=1))
    spp = ctx.enter_context(tc.tile_pool(name="spp", bufs=4, space="PSUM"))
    apsum = ctx.enter_context(tc.tile_pool(name="apsum", bufs=1, space="PSUM"))
    ppsum = ctx.enter_context(tc.tile_pool(name="ppsum", bufs=2, space="PSUM"))
    tmp = ctx.enter_context(tc.tile_pool(name="tmp", bufs=2))
    xpool = ctx.enter_context(tc.tile_pool(name="xpool", bufs=2))
    epool = ctx.enter_context(tc.tile_pool(name="epool", bufs=3))
    ypool = ctx.enter_context(tc.tile_pool(name="ypool", bufs=2))
    rpool = ctx.enter_context(tc.tile_pool(name="rpool", bufs=2))

    # ---- resident SBUF tensors ----
    wq_s = const.tile([P, KO, DG], BF16)
    wk_s = const.tile([P, KO, DG], BF16)
    wv_s = const.tile([P, KO, DG], BF16)
    wo_s = const.tile([P, MT, DM], BF16)
    cos_s = const.tile([P, S], BF16)
    sin_s = const.tile([P, S], BF16)
    rmat_s = const.tile([P, P], BF16)
    tri2_s = const.tile([P, 2, P], F32)
    Qp = const.tile([P, MT, S], BF16)
    Kp = const.tile([P, MT, S], BF16)
    Vs = const.tile([P, NKT, HL, DK + 2], BF16)
    As = const.tile([P, MT, S], BF16)

    # x chunks lead on the sync+scalar queues; weights and the other
    # consts go on the gpsimd+vector queues so x gets the bandwidth.
    for q in range(4):
        ks = slice(2 * q, 2 * q + 2)
        nc.gpsimd.dma_start(wq_s[:, ks, :], d["wqT"][:, ks, :])
    nc.gpsimd.dma_start(rmat_s[:], d["rmat"][:])
    nc.gpsimd.dma_start(cos_s[:, 0 : S // 2], d["cosd"][:, 0 : S // 2])
    nc.gpsimd.dma_start(sin_s[:, 0 : S // 2], d["sind"][:, 0 : S // 2])
    nc.gpsimd.dma_start(wk_s[:], d["wkT"][:])
    nc.gpsimd.dma_start(wv_s[:], d["wvT"][:])
    nc.gpsimd.dma_start(tri2_s[:], d["trimask2"][:])
    nc.gpsimd.dma_start(cos_s[:, S // 2 : S], d["cosd"][:, S // 2 : S])
    nc.gpsimd.dma_start(sin_s[:, S // 2 : S], d["sind"][:, S // 2 : S])
    nc.gpsimd.dma_start(wo_s[:], d["woT"][:])
    # ones column for the denominator rows
    nc.gpsimd.memset(Vs[:, :, :, DK : DK + 1], 1.0)

    # PE p-state warmup: dependency-free matmuls that run while the first
    # DMAs stream in, so real matmuls start at full clock.
    warm = tmp.tile([P, 256], BF16, tag="warm", name="warm", bufs=1)
    nc.vector.memset(warm[:], 0.0)
    for _ in range(45):
        wps = ppsum.tile([P, 256], F32, tag="pp", name="wps")
        nc.tensor.matmul(wps[:], lhsT=warm[:, 0:P], rhs=warm[:],
                         start=True, stop=True)

    def emit_xchunk(b):
        xc = xpool.tile([P, KO, QB], BF16, tag="xc", name="xc")
        for q in range(4):
            ks = slice(2 * q, 2 * q + 2)
            eng = nc.sync if q % 2 == 0 else nc.scalar
            eng.dma_start(xc[:, ks, :], d["xT"][b, :, ks, :])
        return xc

    def emit_qk_piece(xc, b, w_s, dst, mt):
        cols = slice(b * QB, (b + 1) * QB)
        ps = ppsum.tile([P, QB], F32, tag="pp", name="ps")
        for ko in range(KO):
            nc.tensor.matmul(
                ps[:],
                lhsT=(w_s[:, ko, mt * P : (mt + 1) * P]),
                rhs=(xc[:, ko, :]),
                start=(ko == 0),
                stop=(ko == KO - 1),
            )
        qt = tmp.tile([P, QB], BF16, tag="qt", name="qt")
        nc.scalar.copy(qt[:], ps[:])
        # rotation written in place over ps (new accumulation group after
        # the qt cast drains it): one pp-ring slot per piece instead of
        # two, so two projection pieces can be in flight
        nc.tensor.matmul(ps[:], lhsT=(rmat_s[:]), rhs=(qt[:]),
                         start=True, stop=True)
        tsin = tmp.tile([P, QB], BF16, tag="tsin", name="tsin")
        nc.vector.tensor_mul(tsin[:], ps[:], sin_s[:, cols])
        nc.vector.tensor_mul(dst[:, mt, cols], qt[:], cos_s[:, cols])
        nc.vector.tensor_add(dst[:, mt, cols], dst[:, mt, cols], tsin[:])

    def emit_v_piece(xc, b, sq):
        st = 4 * b + sq
        ps = ppsum.tile([P, HL, DK], F32, tag="pp", name="vps")
        for ko in range(KO):
            nc.tensor.matmul(
                ps[:],
                lhsT=(xc[:, ko, sq * P : (sq + 1) * P]),
                rhs=(wv_s[:, ko, :]),
                start=(ko == 0),
                stop=(ko == KO - 1),
            )
        nc.scalar.copy(Vs[:, st, :, 0:DK], ps[:])

    def emit_proj(b, xc):
        with nc.named_scope(f"proj{b}"):
            for w_s, dst in ((wq_s, Qp), (wk_s, Kp)):
                for mt in range(MT):
                    emit_qk_piece(xc, b, w_s, dst, mt)
            for sq in range(4):
                emit_v_piece(xc, b, sq)

    def emit_attn_mt(j, mt):
        jc0 = j * QB
        with nc.named_scope(f"attn{j}_{mt}"):
            ops = {
                h: apsum.tile([P, QB], F32, tag=f"ops{h}", name=f"ops{h}")
                for h in range(2)
            }
            isteps = list(range(4 * j + 4))
            for ch0 in range(0, len(isteps), 3):
                chunk = isteps[ch0 : ch0 + 3]
                sps = []
                for i in chunk:
                    c0 = P * (i - 4 * j) if i >= 4 * j else 0
                    sp = spp.tile([P, 2, QB], F32, tag="sp", name="sp",
                                  bufs=2)
                    for h in range(2):
                        pb = DK * h
                        nc.tensor.matmul(
                            sp[:, h, c0:QB],
                            lhsT=(Kp[pb : pb + DK, mt, i * P : (i + 1) * P]),
                            rhs=(Qp[pb : pb + DK, mt, jc0 + c0 : jc0 + QB]),
                            start=True,
                            stop=True,
                        )
                    sps.append((i, c0, sp))
                for i, c0, sp in sps:
                    if i >= 4 * j:
                        nc.vector.tensor_add(
                            sp[:, :, c0 : c0 + P], sp[:, :, c0 : c0 + P],
                            tri2_s[:],
                        )
                ets = []
                for i, c0, sp in sps:
                    if j == 3 and mt == 0:
                        # ACT is the bottleneck in the last row; exp on DVE
                        # via the Schraudolph bit trick in bf16: bits =
                        # round(s*128*log2(e) + (127*128 - c)) as int16,
                        # reinterpreted. Masked scores (-1e30) saturate to
                        # 0x8000 = -0.0, i.e. weight 0.
                        eti = epool.tile([P, 2, QB], mybir.dt.int16,
                                         tag="et", name="eti")
                        nc.vector.tensor_scalar(
                            out=eti[:, :, c0:QB], in0=sp[:, :, c0:QB],
                            scalar1=184.66496, scalar2=16248.6,
                            op0=mybir.AluOpType.mult,
                            op1=mybir.AluOpType.add,
                        )
                        et = eti.bitcast(BF16)
                    else:
                        et = epool.tile([P, 2, QB], BF16, tag="et", name="et")
                        nc.scalar.activation(et[:, :, c0:QB], sp[:, :, c0:QB],
                                             EXP)
                    ets.append(et)
                for (i, c0, sp), et in zip(sps, ets):
                    for h in range(2):
                        nc.tensor.matmul(
                            ops[h][0 : DK + 1, c0:QB],
                            lhsT=(Vs[:, i, 2 * mt + h, 0 : DK + 1]),
                            rhs=(et[:, h, c0:QB]),
                            start=(i == 0),
                            stop=(i == 4 * j + 3),
                        )
            return ops

    def emit_normalize(j, mt, ops):
        jcols = slice(j * QB, (j + 1) * QB)
        for h in range(2):
            pb = DK * h
            drow = rpool.tile([1, QB], F32, tag="drow", name="drow")
            if mt == 1:
                # the row's exps just drained — ACT has slack here
                nc.scalar.copy(drow[:], ops[h][DK : DK + 1, :])
            else:
                nc.vector.tensor_copy(drow[:], ops[h][DK : DK + 1, :])
            drec = rpool.tile([1, QB], F32, tag="drec", name="drec")
            nc.vector.reciprocal_approx_fast(drec[:], drow[:])
            rb = rpool.tile([DK, QB], F32, tag="rb", name="rb")
            nc.gpsimd.partition_broadcast(rb[:], drec[:], channels=DK)
            nc.vector.tensor_mul(
                As[pb : pb + DK, mt, jcols], ops[h][0:DK, :], rb[:]
            )

    def emit_oproj(j, pre=None):
        with nc.named_scope(f"oproj{j}"):
            for sq in range(4):
                st = 4 * j + sq
                for nh2 in range(2):
                    if j == 3:
                        # the pp ring is free in the last row (no proj(4)),
                        # and unlike the ops tags it has no WAR on the
                        # row's final attention accumulators
                        yps = ppsum.tile([P, QB], F32, tag="pp", name="yps")
                    else:
                        yps = apsum.tile([P, QB], F32, tag=f"ops{nh2}",
                                         name="yps")
                    for p_ in range(MT):
                        nc.tensor.matmul(
                            yps[:],
                            lhsT=(As[:, p_, st * P : (st + 1) * P]),
                            rhs=(wo_s[:, p_, nh2 * QB : (nh2 + 1) * QB]),
                            start=(p_ == 0),
                            stop=(p_ == MT - 1),
                        )
                    ysb = ypool.tile([P, QB], BF16, tag="ysb", name="ysb",
                                     bufs=3)
                    if nh2 == 0:
                        nc.scalar.copy(ysb[:], yps[:])
                    else:
                        nc.vector.tensor_copy(ysb[:], yps[:])
                    if j == 3:
                        eng = (nc.sync, nc.gpsimd, nc.scalar)[(2 * sq + nh2) % 3]
                    else:
                        eng = nc.sync if nh2 == 0 else nc.gpsimd
                    eng.dma_start(d["y"][st, nh2], ysb[:])

    # ---- software-pipelined emission ----
    xcs = [emit_xchunk(0), emit_xchunk(1)]
    emit_proj(0, xcs[0])
    for j in range(NQB):
        if j + 2 < NQB:
            xcs.append(emit_xchunk(j + 2))
        for mt in range(MT):
            ops = emit_attn_mt(j, mt)
            emit_normalize(j, mt, ops)
        emit_oproj(j)
        if j + 1 < NQB:
            emit_proj(j + 1, xcs[j + 1])


def _build():
    nc = bacc.Bacc("TRN2", target_bir_lowering=False, debug=False,
                   num_devices=N_CORES)
    d = {}
    d["xT"] = nc.dram_tensor("xT", [NQB, P, KO, QB], BF16, kind="ExternalInput").ap()
    d["wqT"] = nc.dram_tensor("wqT", [P, KO, DG], BF16, kind="ExternalInput").ap()
    d["wkT"] = nc.dram_tensor("wkT", [P, KO, DG], BF16, kind="ExternalInput").ap()
    d["wvT"] = nc.dram_tensor("wvT", [P, KO, DG], BF16, kind="ExternalInput").ap()
    d["woT"] = nc.dram_tensor("woT", [P, MT, DM], BF16, kind="ExternalInput").ap()
    d["cosd"] = nc.dram_tensor("cosd", [P, S], BF16, kind="ExternalInput").ap()
    d["sind"] = nc.dram_tensor("sind", [P, S], BF16, kind="ExternalInput").ap()
    d["rmat"] = nc.dram_tensor("rmat", [P, P], BF16, kind="ExternalInput").ap()
    d["trimask2"] = nc.dram_tensor("trimask2", [P, 2, P], F32, kind="ExternalInput").ap()
    d["y"] = nc.dram_tensor("y", [NKT, 2, P, QB], BF16, kind="ExternalOutput").ap()
    from contextlib import ExitStack
    with tile.TileContext(nc) as tc, ExitStack() as ctx:
        _emit(ctx, tc, d)
    nc.compile()
    return nc


_cache = {}


def _get_nc():
    if "nc" not in _cache:
        _cache["nc"] = _build()
    return _cache["nc"]


def _host_prep(x, token_positions, wq, wk, wv, wo):
    BF = ml_dtypes.bfloat16
    x = np.asarray(x, dtype=np.float32)
    pos = np.asarray(token_positions, dtype=np.float32)
    wq = np.asarray(wq, dtype=np.float32)
    wk = np.asarray(wk, dtype=np.float32)
    wv = np.asarray(wv, dtype=np.float32)
    wo = np.asarray(wo, dtype=np.float32)

    freqs = 1.0 / THETA ** (np.arange(0, DK, 2, dtype=np.float32) / DK)  # (32,)
    ang = pos[:, None] * freqs[None, :]          # (S, 32)
    cos_t, sin_t = np.cos(ang), np.sin(ang)       # (S, 32)
    jmap = (np.arange(P) % DK) // 2               # row -> freq index
    cosd = np.ascontiguousarray(cos_t.T[jmap, :]).astype(BF)  # (128, S)
    sind = np.ascontiguousarray(sin_t.T[jmap, :]).astype(BF)

    rmat = np.zeros((P, P), dtype=np.float32)
    m = np.arange(0, P, 2)
    rmat[m + 1, m] = -1.0   # out[2m]   = -in[2m+1]
    rmat[m, m + 1] = 1.0    # out[2m+1] =  in[2m]
    rmat = rmat.astype(BF)

    tri = np.where(
        np.arange(P)[:, None] <= np.arange(P)[None, :], 0.0, -1e30
    ).astype(np.float32)
    tri2 = np.stack([tri, tri], axis=1)          # (128, 2, 128)

    def tile3(a2d, inner=P):
        # [K, M] -> [inner, K//inner, M] with K = ko*inner + ki
        K, M = a2d.shape
        return np.ascontiguousarray(
            a2d.reshape(K // inner, inner, M).transpose(1, 0, 2)
        )

    in_maps = []
    scale = 1.0 / np.sqrt(np.float32(DK))
    for c in range(N_CORES):
        b, g = divmod(c, HG)
        gs = slice(g * DG, (g + 1) * DG)
        xT = np.ascontiguousarray(
            tile3(x[b].T).reshape(P, KO, NQB, QB).transpose(2, 0, 1, 3)
        ).astype(BF)                                        # [4, 128, 8, 512]
        wqT = tile3((wq[gs] * scale).T.copy()).astype(BF)  # [128, 8, 256]
        wkT = tile3(wk[gs].T.copy()).astype(BF)
        wvT = tile3(wv[gs].T.copy()).astype(BF)
        woT = tile3(wo[:, gs].T.copy()).astype(BF)         # [128, 2, 1024]
        in_maps.append({
            "xT": xT, "wqT": wqT, "wkT": wkT, "wvT": wvT, "woT": woT,
            "cosd": cosd, "sind": sind, "rmat": rmat, "trimask2": tri2,
        })
    return in_maps


def run(x, token_positions, wq, wk, wv, wo, trace=False):
    nc = _get_nc()
    in_maps = _host_prep(x, token_positions, wq, wk, wv, wo)
    res = run_bass_kernel_spmd(nc, in_maps, list(range(N_CORES)), trace=trace)
    y = np.zeros((B, S, DM), dtype=np.float32)
    for c in range(N_CORES):
        blk = np.asarray(res.results[c]["y"], dtype=np.float32)
        y[c // HG] += blk.transpose(0, 2, 1, 3).reshape(S, DM)
    return y, res


def kernel(x, token_positions, wq, wk, wv, wo):
    y, _ = run(x, token_positions, wq, wk, wv, wo)
    return y


# revision 105
# speedup vs baseline: 1.0202x; 1.0202x over previous
"""Multi-head self-attention with RoPE on 8 Trainium2 NeuronCores.

Sharding: data-parallel over batch (2) x tensor-parallel over heads
(16 heads -> 4 groups of 4). Core c handles batch c//4, head group c%4.
Each core computes a partial output projection (d_in-sharded wo); the
4 partials per batch are summed on the host (the unshard step).

Optimized around the PE p-state ramp: the PE runs at 2.4 GHz only after
~3us of continuous busy, so the schedule keeps an uninterrupted PE
instruction stream:
  - Projections (fp32r) and attention (bf16) use SEPARATE psum rings so
    projection matmuls for chunk b+1 are always ready as filler while
    the attention exp pipeline (ACT engine) catches up.
  - Scores for a head pair land in one 2-bank psum tile [128, 2, 512],
    so each k-tile needs a single merged Exp activation.
  - Attention tensors (Q/K post-rope, V, exp scores, normalized A, wo)
    are bf16: <256-col band matmuls run at 1 cyc/row, DVE ops get 2x/4x
    modes, LDWEIGHTS halves.
  - Emission order is software-pipelined: B(j) attention, then A(j+1)
    projections, then C(j) output projection; the tile scheduler fills
    PE stalls with whatever is ready.
Per-core math identical to the baseline: scores computed transposed
S[k, q] = K' Q'^T; V carries a ones column so the attention-out matmul
accumulates the softmax denominator for free; no max-subtraction.
"""

import sys

for _p in ("/opt/trn_rl_repo", "/opt/pypackages"):
    if _p not in sys.path:
        sys.path.append(_p)

import numpy as np
import ml_dtypes

import concourse.bass as bass
from concourse.bass import _add_dep_helper
import concourse.mybir as mybir
import concourse.tile as tile
from concourse import bacc
from concourse.bass_utils import run_bass_kernel_spmd

# Problem constants (hardcoded per contract)
B = 2
S = 2048
DM = 1024
NH = 16
DK = 64
THETA = 10000.0
N_CORES = 8
HG = 4            # head groups (tensor-parallel)
HL = NH // HG     # heads per core = 4
DG = HL * DK      # group out dim = 256

P = 128
KO = DM // P      # 8 contraction subtiles for projections
MT = 2            # 128-row tiles of the 256-wide Q/K head-group dim
QB = 512          # q block width
NQB = S // QB     # 4
NKT = S // P      # 16 k tiles
F32 = mybir.dt.float32
F32R = mybir.dt.float32r
BF16 = mybir.dt.bfloat16
EXP = mybir.ActivationFunctionType.Exp


def _emit(ctx, tc, d):
    nc = tc.nc
    # PSUM: spp (scores, [128,2,512] = 2 banks) x2 = 4 banks;
    # ops0/ops1 (attention accumulators + oproj outputs) = 2 banks;
    # pp (projection transients) x2 = 2 banks. Total 8.
    const = ctx.enter_context(tc.tile_pool(name="const", bufs=1))
    spp = ctx.enter_context(tc.tile_pool(name="spp", bufs=4, space="PSUM"))
    apsum = ctx.enter_context(tc.tile_pool(name="apsum", bufs=1, space="PSUM"))
    ppsum = ctx.enter_context(tc.tile_pool(name="ppsum", bufs=2, space="PSUM"))
    tmp = ctx.enter_context(tc.tile_pool(name="tmp", bufs=2))
    xpool = ctx.enter_context(tc.tile_pool(name="xpool", bufs=2))
    epool = ctx.enter_context(tc.tile_pool(name="epool", bufs=3))
    ypool = ctx.enter_context(tc.tile_pool(name="ypool", bufs=2))
    rpool = ctx.enter_context(tc.tile_pool(name="rpool", bufs=2))

    # ---- resident SBUF tensors ----
    wq_s = const.tile([P, KO, DG], BF16)
    wk_s = const.tile([P, KO, DG], BF16)
    wv_s = const.tile([P, KO, DG], BF16)
    wo_s = const.tile([P, MT, DM], BF16)
    cos_s = const.tile([P, S], BF16)
    sin_s = const.tile([P, S], BF16)
    rmat_s = const.tile([P, P], BF16)
    tri2_s = const.tile([P, 2, P], F32)
    Qp = const.tile([P, MT, S], BF16)
    Kp = const.tile([P, MT, S], BF16)
    Vs = const.tile([P, NKT, HL, DK + 2], BF16)
    As = const.tile([P, MT, S], BF16)

    # x chunks lead on the sync+scalar queues; weights and the other
    # consts go on the gpsimd+vector queues so x gets the bandwidth.
    for q in range(4):
        ks = slice(2 * q, 2 * q + 2)
        nc.gpsimd.dma_start(wq_s[:, ks, :], d["wqT"][:, ks, :])
    nc.gpsimd.dma_start(rmat_s[:], d["rmat"][:])
    nc.gpsimd.dma_start(cos_s[:, 0 : S // 2], d["cosd"][:, 0 : S // 2])
    nc.gpsimd.dma_start(sin_s[:, 0 : S // 2], d["sind"][:, 0 : S // 2])
    nc.gpsimd.dma_start(wk_s[:], d["wkT"][:])
    nc.gpsimd.dma_start(wv_s[:], d["wvT"][:])
    nc.gpsimd.dma_start(tri2_s[:], d["trimask2"][:])
    nc.gpsimd.dma_start(cos_s[:, S // 2 : S], d["cosd"][:, S // 2 : S])
    nc.gpsimd.dma_start(sin_s[:, S // 2 : S], d["sind"][:, S // 2 : S])
    nc.gpsimd.dma_start(wo_s[:], d["woT"][:])
    # ones column for the denominator rows
    nc.gpsimd.memset(Vs[:, :, :, DK : DK + 1], 1.0)

    # PE p-state warmup: dependency-free matmuls that run while the first
    # DMAs stream in, so real matmuls start at full clock.
    warm = tmp.tile([P, 256], BF16, tag="warm", name="warm", bufs=1)
    nc.vector.memset(warm[:], 0.0)
    for _ in range(45):
        wps = ppsum.tile([P, 256], F32, tag="pp", name="wps")
        nc.tensor.matmul(wps[:], lhsT=warm[:, 0:P], rhs=warm[:],
                         start=True, stop=True)

    def emit_xchunk(b):
        xc = xpool.tile([P, KO, QB], BF16, tag="xc", name="xc")
        for q in range(4):
            ks = slice(2 * q, 2 * q + 2)
            eng = nc.sync if q % 2 == 0 else nc.scalar
            eng.dma_start(xc[:, ks, :], d["xT"][b, :, ks, :])
        return xc

    def emit_qk_piece(xc, b, w_s, dst, mt):
        cols = slice(b * QB, (b + 1) * QB)
        ps = ppsum.tile([P, QB], F32, tag="pp", name="ps")
        for ko in range(KO):
            nc.tensor.matmul(
                ps[:],
                lhsT=(w_s[:, ko, mt * P : (mt + 1) * P]),
                rhs=(xc[:, ko, :]),
                start=(ko == 0),
                stop=(ko == KO - 1),
            )
        qt = tmp.tile([P, QB], BF16, tag="qt", name="qt", bufs=3)
        nc.scalar.copy(qt[:], ps[:])
        # rotation written in place over ps (new accumulation group after
        # the qt cast drains it): one pp-ring slot per piece instead of
        # two, so two projection pieces can be in flight
        nc.tensor.matmul(ps[:], lhsT=(rmat_s[:]), rhs=(qt[:]),
                         start=True, stop=True)
        tsin = tmp.tile([P, QB], BF16, tag="tsin", name="tsin", bufs=3)
        nc.vector.tensor_mul(tsin[:], ps[:], sin_s[:, cols])
        nc.vector.tensor_mul(dst[:, mt, cols], qt[:], cos_s[:, cols])
        nc.vector.tensor_add(dst[:, mt, cols], dst[:, mt, cols], tsin[:])

    def emit_v_piece(xc, b, sq):
        st = 4 * b + sq
        ps = ppsum.tile([P, HL, DK], F32, tag="pp", name="vps")
        for ko in range(KO):
            nc.tensor.matmul(
                ps[:],
                lhsT=(xc[:, ko, sq * P : (sq + 1) * P]),
                rhs=(wv_s[:, ko, :]),
                start=(ko == 0),
                stop=(ko == KO - 1),
            )
        nc.scalar.copy(Vs[:, st, :, 0:DK], ps[:])

    def emit_proj(b, xc):
        with nc.named_scope(f"proj{b}"):
            for w_s, dst in ((wq_s, Qp), (wk_s, Kp)):
                for mt in range(MT):
                    emit_qk_piece(xc, b, w_s, dst, mt)
            for sq in range(4):
                emit_v_piece(xc, b, sq)

    def emit_attn_mt(j, mt):
        jc0 = j * QB
        with nc.named_scope(f"attn{j}_{mt}"):
            ops = {
                h: apsum.tile([P, QB], F32, tag=f"ops{h}", name=f"ops{h}")
                for h in range(2)
            }
            isteps = list(range(4 * j + 4))
            for ch0 in range(0, len(isteps), 4):
                chunk = isteps[ch0 : ch0 + 4]
                sps = []
                for i in chunk:
                    c0 = P * (i - 4 * j) if i >= 4 * j else 0
                    sp = spp.tile([P, 2, QB], F32, tag="sp", name="sp",
                                  bufs=2)
                    for h in range(2):
                        pb = DK * h
                        nc.tensor.matmul(
                            sp[:, h, c0:QB],
                            lhsT=(Kp[pb : pb + DK, mt, i * P : (i + 1) * P]),
                            rhs=(Qp[pb : pb + DK, mt, jc0 + c0 : jc0 + QB]),
                            start=True,
                            stop=True,
                        )
                    sps.append((i, c0, sp))
                for i, c0, sp in sps:
                    if i >= 4 * j:
                        nc.vector.tensor_add(
                            sp[:, :, c0 : c0 + P], sp[:, :, c0 : c0 + P],
                            tri2_s[:],
                        )
                ets = []
                for i, c0, sp in sps:
                    if j == 3 and mt == 0:
                        # ACT is the bottleneck in the last row; exp on DVE
                        # via the Schraudolph bit trick in bf16: bits =
                        # round(s*128*log2(e) + (127*128 - c)) as int16,
                        # reinterpreted. Masked scores (-1e30) saturate to
                        # 0x8000 = -0.0, i.e. weight 0.
                        eti = epool.tile([P, 2, QB], mybir.dt.int16,
                                         tag="et", name="eti")
                        nc.vector.tensor_scalar(
                            out=eti[:, :, c0:QB], in0=sp[:, :, c0:QB],
                            scalar1=184.66496, scalar2=16248.6,
                            op0=mybir.AluOpType.mult,
                            op1=mybir.AluOpType.add,
                        )
                        et = eti.bitcast(BF16)
                    else:
                        et = epool.tile([P, 2, QB], BF16, tag="et", name="et")
                        nc.scalar.activation(et[:, :, c0:QB], sp[:, :, c0:QB],
                                             EXP)
                    ets.append(et)
                for (i, c0, sp), et in zip(sps, ets):
                    for h in range(2):
                        nc.tensor.matmul(
                            ops[h][0 : DK + 1, c0:QB],
                            lhsT=(Vs[:, i, 2 * mt + h, 0 : DK + 1]),
                            rhs=(et[:, h, c0:QB]),
                            start=(i == 0),
                            stop=(i == 4 * j + 3),
                        )
            return ops

    def emit_normalize(j, mt, ops):
        jcols = slice(j * QB, (j + 1) * QB)
        for h in range(2):
            pb = DK * h
            drow = rpool.tile([1, QB], F32, tag="drow", name="drow")
            if mt == 1:
                # the row's exps just drained — ACT has slack here
                nc.scalar.copy(drow[:], ops[h][DK : DK + 1, :])
            else:
                nc.vector.tensor_copy(drow[:], ops[h][DK : DK + 1, :])
            drec = rpool.tile([1, QB], F32, tag="drec", name="drec")
            nc.vector.reciprocal_approx_fast(drec[:], drow[:])
            rb = rpool.tile([DK, QB], F32, tag="rb", name="rb", bufs=3)
            nc.gpsimd.partition_broadcast(rb[:], drec[:], channels=DK)
            nc.vector.tensor_mul(
                As[pb : pb + DK, mt, jcols], ops[h][0:DK, :], rb[:]
            )

    def emit_oproj(j, pre=None):
        with nc.named_scope(f"oproj{j}"):
            for sq in range(4):
                st = 4 * j + sq
                for nh2 in range(2):
                    if j == 3:
                        # the pp ring is free in the last row (no proj(4)),
                        # and unlike the ops tags it has no WAR on the
                        # row's final attention accumulators
                        yps = ppsum.tile([P, QB], F32, tag="pp", name="yps")
                    else:
                        yps = apsum.tile([P, QB], F32, tag=f"ops{nh2}",
                                         name="yps")
                    for p_ in range(MT):
                        nc.tensor.matmul(
                            yps[:],
                            lhsT=(As[:, p_, st * P : (st + 1) * P]),
                            rhs=(wo_s[:, p_, nh2 * QB : (nh2 + 1) * QB]),
                            start=(p_ == 0),
                            stop=(p_ == MT - 1),
                        )
                    ysb = ypool.tile([P, QB], BF16, tag="ysb", name="ysb",
                                     bufs=3)
                    if nh2 == 0:
                        nc.scalar.copy(ysb[:], yps[:])
                    else:
                        nc.vector.tensor_copy(ysb[:], yps[:])
                    if j == 3:
                        eng = (nc.sync, nc.gpsimd, nc.scalar)[(2 * sq + nh2) % 3]
                    else:
                        # keep triggers off gpsimd, whose queue carries the
                        # normalize-critical partition broadcasts
                        eng = nc.sync
                    eng.dma_start(d["y"][st, nh2], ysb[:])

    # ---- software-pipelined emission ----
    xcs = [emit_xchunk(0), emit_xchunk(1)]
    emit_proj(0, xcs[0])
    for j in range(NQB):
        if j + 2 < NQB:
            xcs.append(emit_xchunk(j + 2))
        for mt in range(MT):
            ops = emit_attn_mt(j, mt)
            emit_normalize(j, mt, ops)
        emit_oproj(j)
        if j + 1 < NQB:
            emit_proj(j + 1, xcs[j + 1])


def _build():
    nc = bacc.Bacc("TRN2", target_bir_lowering=False, debug=False,
                   num_devices=N_CORES)
    d = {}
    d["xT"] = nc.dram_tensor("xT", [NQB, P, KO, QB], BF16, kind="ExternalInput").ap()
    d["wqT"] = nc.dram_tensor("wqT", [P, KO, DG], BF16, kind="ExternalInput").ap()
    d["wkT"] = nc.dram_tensor("wkT", [P, KO, DG], BF16, kind="ExternalInput").ap()
    d["wvT"] = nc.dram_tensor("wvT", [P, KO, DG], BF16, kind="ExternalInput").ap()
    d["woT"] = nc.dram_tensor("woT", [P, MT, DM], BF16, kind="ExternalInput").ap()
    d["cosd"] = nc.dram_tensor("cosd", [P, S], BF16, kind="ExternalInput").ap()
    d["sind"] = nc.dram_tensor("sind", [P, S], BF16, kind="ExternalInput").ap()
    d["rmat"] = nc.dram_tensor("rmat", [P, P], BF16, kind="ExternalInput").ap()
    d["trimask2"] = nc.dram_tensor("trimask2", [P, 2, P], F32, kind="ExternalInput").ap()
    d["y"] = nc.dram_tensor("y", [NKT, 2, P, QB], BF16, kind="ExternalOutput").ap()
    from contextlib import ExitStack
    with tile.TileContext(nc) as tc, ExitStack() as ctx:
        _emit(ctx, tc, d)
    nc.compile()
    return nc


_cache = {}


def _get_nc():
    if "nc" not in _cache:
        _cache["nc"] = _build()
    return _cache["nc"]


def _host_prep(x, token_positions, wq, wk, wv, wo):
    BF = ml_dtypes.bfloat16
    x = np.asarray(x, dtype=np.float32)
    pos = np.asarray(token_positions, dtype=np.float32)
    wq = np.asarray(wq, dtype=np.float32)
    wk = np.asarray(wk, dtype=np.float32)
    wv = np.asarray(wv, dtype=np.float32)
    wo = np.asarray(wo, dtype=np.float32)

    freqs = 1.0 / THETA ** (np.arange(0, DK, 2, dtype=np.float32) / DK)  # (32,)
    ang = pos[:, None] * freqs[None, :]          # (S, 32)
    cos_t, sin_t = np.cos(ang), np.sin(ang)       # (S, 32)
    jmap = (np.arange(P) % DK) // 2               # row -> freq index
    cosd = np.ascontiguousarray(cos_t.T[jmap, :]).astype(BF)  # (128, S)
    sind = np.ascontiguousarray(sin_t.T[jmap, :]).astype(BF)

    rmat = np.zeros((P, P), dtype=np.float32)
    m = np.arange(0, P, 2)
    rmat[m + 1, m] = -1.0   # out[2m]   = -in[2m+1]
    rmat[m, m + 1] = 1.0    # out[2m+1] =  in[2m]
    rmat = rmat.astype(BF)

    tri = np.where(
        np.arange(P)[:, None] <= np.arange(P)[None, :], 0.0, -1e30
    ).astype(np.float32)
    tri2 = np.stack([tri, tri], axis=1)          # (128, 2, 128)

    def tile3(a2d, inner=P):
        # [K, M] -> [inner, K//inner, M] with K = ko*inner + ki
        K, M = a2d.shape
        return np.ascontiguousarray(
            a2d.reshape(K // inner, inner, M).transpose(1, 0, 2)
        )

    in_maps = []
    scale = 1.0 / np.sqrt(np.float32(DK))
    for c in range(N_CORES):
        b, g = divmod(c, HG)
        gs = slice(g * DG, (g + 1) * DG)
        xT = np.ascontiguousarray(
            tile3(x[b].T).reshape(P, KO, NQB, QB).transpose(2, 0, 1, 3)
        ).astype(BF)                                        # [4, 128, 8, 512]
        wqT = tile3((wq[gs] * scale).T.copy()).astype(BF)  # [128, 8, 256]
        wkT = tile3(wk[gs].T.copy()).astype(BF)
        wvT = tile3(wv[gs].T.copy()).astype(BF)
        woT = tile3(wo[:, gs].T.copy()).astype(BF)         # [128, 2, 1024]
        in_maps.append({
            "xT": xT, "wqT": wqT, "wkT": wkT, "wvT": wvT, "woT": woT,
            "cosd": cosd, "sind": sind, "rmat": rmat, "trimask2": tri2,
        })
    return in_maps


def run(x, token_positions, wq, wk, wv, wo, trace=False):
    nc = _get_nc()
    in_maps = _host_prep(x, token_positions, wq, wk, wv, wo)
    res = run_bass_kernel_spmd(nc, in_maps, list(range(N_CORES)), trace=trace)
    y = np.zeros((B, S, DM), dtype=np.float32)
    for c in range(N_CORES):
        blk = np.asarray(res.results[c]["y"], dtype=np.float32)
        y[c // HG] += blk.transpose(0, 2, 1, 3).reshape(S, DM)
    return y, res


def kernel(x, token_positions, wq, wk, wv, wo):
    y, _ = run(x, token_positions, wq, wk, wv, wo)
    return y
